# revision 19
# baseline (speedup 1.0000x reference)
"""Trainium2 Bass kernel for nn_C2f_DualModal_MoE — v2 (mixed precision).

Full inputs in, full outputs out. Data-parallel over batch: 16 items on
8 cores (2 per core). Per-core numerics:
  - cv1 / cv2 1x1 convs in bf16 (same PE rate as f32r, half DMA/SBUF)
  - expert 3x3 convs in fp8e4m3 with DoubleRow matmuls: the padded image
    is stored flat [C, 82*82] so every conv tap is a contiguous 410-col
    window; taps are paired into 5 DoubleRow matmuls (2x contraction at
    0.5 cyc/row -> ~3.5x the f32r expert throughput).
  - routing (softmax top-2, gates) in f32 on device; selected experts'
    weights gathered by dynamic-offset DMA from an fp8 pair-blob.

Spatial work is processed in PAIRS of 5-row blocks: each psum tile
[C, 1024] holds the same output-chunk for two consecutive blocks at
bank-aligned halves, so every silu is one wide [C, 2, n] op with a
legal scalar (per-partition) bias. Two psum tags x 2 bufs = exactly
8 banks, shared by the cv1 and expert/cv2 phases.

Schedule per core (items b=0,1): p1(0) -> p1(1) [rt(0) hidden inside] ->
p2(0) [rt(1) hidden inside] -> p2(1), with a 1-pair software pipeline
in p2 (expert matmuls of pair j overlap cv2 of pair j-1).
"""

import sys

for _p in ("/opt/trn_rl_repo", "/opt/pypackages"):
    if _p not in sys.path:
        sys.path.append(_p)

import numpy as np
import ml_dtypes
import concourse.bass as bass
import concourse.mybir as mybir
import concourse.tile as tile
from concourse import bacc
from concourse.bass import ds
from concourse.bass_utils import run_bass_kernel_spmd

F32 = mybir.dt.float32
F32R = mybir.dt.float32r
BF16 = mybir.dt.bfloat16
FP8 = mybir.dt.float8e4
U32 = mybir.dt.uint32
AF = mybir.ActivationFunctionType
DR = mybir.MatmulPerfMode.DoubleRow
AX = mybir.AxisListType.X

N_CORES = 8
B = 16
BPC = B // N_CORES
C1 = 256
C = 128
E = 4
H = W = 80
S = H * W  # 6400
HP = H + 2  # 82
R = 5  # image rows per spatial block
NT = H // R  # 16 blocks
NP = NT // 2  # 8 block pairs
N = R * W  # 400 dense cols per block
NW = R * HP  # 410 padded-window cols per block
MLEN = 1 + HP * HP + 7  # flat fp8 padded image + guard cells
SLEN = HP * HP  # flat expert-output buffer (padded coords)
# DoubleRow tap pairs: (offA, offB) flat offsets rel. to the output
# position; off(dy,dx) = (dy-1)*HP + (dx-1). Last pair's B is a dummy
# (zero weights) one past tap8.
PAIRS = ((-83, -82), (-81, -1), (0, 1), (81, 82), (83, 84))
TAP_PAIRS = (((0, 0), (0, 1)), ((0, 2), (1, 0)), ((1, 1), (1, 2)),
             ((2, 0), (2, 1)), ((2, 2), None))
WCOLS = 5 * 2 * C  # 1280 fp8 weight cols per expert
INV_S = 1.0 / S

_cache = {}


def _ap(view, extra_off, dims):
    """Manual AP at view.offset + extra_off with [stride, size] dims."""
    return bass.AP(view.tensor, view.offset + extra_off, [list(d) for d in dims])


def _build_program(reps=1):
    nc = bacc.Bacc(
        "TRN2",
        target_bir_lowering=False,
        debug=False,
        enable_asserts=True,
        dynamic_dma_scratch_size=4096,
    )
    x_d = nc.dram_tensor("xb", [BPC, C1, S], BF16, kind="ExternalInput").ap()
    w1_d = nc.dram_tensor("w1t", [C1, 2 * C], BF16, kind="ExternalInput").ap()
    b1_d = nc.dram_tensor("b1", [2 * C, 1], F32, kind="ExternalInput").ap()
    wr_d = nc.dram_tensor("wrt", [C, E], F32, kind="ExternalInput").ap()
    br_d = nc.dram_tensor("br", [E, 1], F32, kind="ExternalInput").ap()
    wexp_d = nc.dram_tensor("wexp8", [E * C, WCOLS], FP8, kind="ExternalInput").ap()
    bexp_d = nc.dram_tensor("bexp", [E * C, 1], F32, kind="ExternalInput").ap()
    w2_d = nc.dram_tensor("w2t", [3 * C, 2 * C], BF16, kind="ExternalInput").ap()
    b2_d = nc.dram_tensor("b2", [2 * C, 1], F32, kind="ExternalInput").ap()
    out_d = nc.dram_tensor("out", [BPC, 2 * C, S], F32, kind="ExternalOutput").ap()

    with tile.TileContext(nc) as tc:
        _emit(nc, tc, x_d, w1_d, b1_d, wr_d, br_d, wexp_d, bexp_d, w2_d, b2_d,
              out_d, reps)
    nc.compile()
    return nc


def _emit(nc, tc, x_d, w1_d, b1_d, wr_d, br_d, wexp_d, bexp_d, w2_d, b2_d,
          out_d, reps=1):
    from contextlib import ExitStack

    ctx = ExitStack()
    with ctx:
        wp = ctx.enter_context(tc.tile_pool(name="weights", bufs=1))
        sp = ctx.enter_context(tc.tile_pool(name="stream", bufs=2))
        pp = ctx.enter_context(tc.tile_pool(name="psum", bufs=1, space="PSUM"))

        # cv1 weights first: critical path to the first matmul. Split them
        # across the sync and pool queues (the scalar queue is blocked by the
        # act-table preload at startup).
        w1k = []
        for k in range(2):
            t = wp.tile([C, 2 * C], BF16, name=f"w1k{k}")
            eng = nc.sync if k == 0 else nc.gpsimd
            eng.dma_start(t[:], w1_d[k * C:(k + 1) * C, :])
            w1k.append(t)
        b1sb = wp.tile([C, 2], F32, name="b1sb")
        for mt in range(2):
            nc.gpsimd.dma_start(b1sb[:, mt:mt + 1], b1_d[mt * C:(mt + 1) * C, :])

        moe_fl = []
        for b in range(BPC):
            t = wp.tile([C, SLEN], BF16, name=f"moe_{b}")
            moe_fl.append(t)

        # flat fp8 padded images, borders zeroed once
        m8 = []
        for b in range(BPC):
            t = wp.tile([C, MLEN], FP8, name=f"m8_{b}")
            nc.gpsimd.memset(t[:, 0:84], 0.0)
            nc.gpsimd.memset(
                _ap(t[:], 164, [[MLEN, C], [HP, 80], [1, 2]]), 0.0)
            nc.gpsimd.memset(t[:, 6644:MLEN], 0.0)
            m8.append(t)

        def setup_tail():
            w2c = []
            for j in range(3):
                t = wp.tile([C, 2 * C], BF16, name=f"w2c{j}")
                nc.gpsimd.dma_start(t[:], w2_d[j * C:(j + 1) * C, :])
                w2c.append(t)
            b2sb = wp.tile([C, 2], F32, name="b2sb")
            for mt in range(2):
                nc.gpsimd.dma_start(b2sb[:, mt:mt + 1], b2_d[mt * C:(mt + 1) * C, :])
            wrtsb = wp.tile([C, E], F32, name="wrtsb")
            nc.gpsimd.dma_start(wrtsb[:], wr_d[:])
            brsb = wp.tile([E, 1], F32, name="brsb")
            nc.gpsimd.dma_start(brsb[:], br_d[:])
            ones = wp.tile([1, C], F32, name="ones")
            nc.vector.memset(ones[:], 1.0)
            return w2c, b2sb, wrtsb, brsb, ones

        def p1_state(b):
            ym = sp.tile([C, 2, S], BF16, tag="ym", bufs=2, name=f"ym{b}")
            parts = sp.tile([C, NP], F32, tag="parts", bufs=2, name=f"parts{b}")
            return ym, parts

        def p1_pair(b, st, j, defer_post=False):
            ym, parts = st
            w0 = 2 * j * N  # first dense col of the pair (800 cols)
            xw = []
            for k in range(2):
                xt = sp.tile([C, 2 * N], BF16, tag=f"x{k}", bufs=3,
                             name=f"x{k}_{j}")
                nc.sync.dma_start(
                    xt[:], x_d[b, k * C:(k + 1) * C, w0:w0 + 2 * N])
                xw.append(xt)
            psA = pp.tile([C, 1024], F32, tag="TA", bufs=2, name=f"psA{j}")
            psM = pp.tile([C, 1024], F32, tag="TB", bufs=2, name=f"psM{j}")
            for blk in range(2):
                for k in range(2):
                    nc.tensor.matmul(
                        psA[:, blk * 512:blk * 512 + N],
                        w1k[k][:, 0:C],
                        xw[k][:, blk * N:(blk + 1) * N],
                        start=(k == 0), stop=(k == 1))
                    nc.tensor.matmul(
                        psM[:, blk * 512:blk * 512 + N],
                        w1k[k][:, C:2 * C],
                        xw[k][:, blk * N:(blk + 1) * N],
                        start=(k == 0), stop=(k == 1))

            def post():
                pinA = _ap(psA, 0, [[1024, C], [512, 2], [1, N]])
                pinM = _ap(psM, 0, [[1024, C], [512, 2], [1, N]])
                nc.scalar.activation(
                    ym[:, 0, w0:w0 + 2 * N], pinA, AF.Silu, bias=b1sb[:, 0:1])
                nc.scalar.activation(
                    ym[:, 1, w0:w0 + 2 * N], pinM, AF.Silu, bias=b1sb[:, 1:2])
                # fp8 copy of the m half into the padded flat image
                msrc = _ap(ym[:], S + w0, [[2 * S, C], [W, 2 * R], [1, W]])
                mdst = _ap(m8[b][:], 1 + (2 * R * j + 1) * HP + 1,
                           [[MLEN, C], [HP, 2 * R], [1, W]])
                nc.vector.tensor_copy(mdst, msrc)
                # partial spatial sum of m for the router
                nc.vector.reduce_sum(
                    parts[:, j:j + 1], ym[:, 1, w0:w0 + 2 * N], axis=AX)

            if defer_post:
                return post
            post()

        def p1_pairs(b, st, j0, j1):
            for j in range(j0, j1):
                p1_pair(b, st, j)

        def routing(b, st):
            _, parts = st
            pooled = sp.tile([C, 1], F32, tag="pooled", bufs=2)
            nc.vector.reduce_sum(pooled[:], parts[:], axis=AX)
            ps_l = pp.tile([E, 1], F32, tag="TB", bufs=2, name="ps_l")
            nc.tensor.matmul(ps_l[:], wrtsb[:], pooled[:], start=True, stop=True)
            l_sb = sp.tile([E, 1], F32, tag="lsb", bufs=2)
            nc.vector.tensor_scalar(
                l_sb[:], ps_l[:], INV_S, None, op0=mybir.AluOpType.mult)
            nc.vector.tensor_tensor(l_sb[:], l_sb[:], brsb[:],
                                    op=mybir.AluOpType.add)
            row = sp.tile([1, 8], F32, tag="row", bufs=2)
            nc.vector.memset(row[:], -1e30)
            nc.gpsimd.dma_start(row[0:1, 0:E], l_sb[0:E, 0:1])
            vals = sp.tile([1, 8], F32, tag="vals", bufs=2)
            nc.vector.max(vals[:], row[:])
            uidx = sp.tile([1, 8], U32, tag="uidx", bufs=2)
            nc.vector.max_index(uidx[:], vals[:], row[:])
            # g0 = sigmoid(l0 - l1) = silu(d)/d, g1 = 1 - g0
            scr = sp.tile([1, 4], F32, tag="scr", bufs=2)
            nc.vector.tensor_tensor(scr[:, 0:1], vals[:, 0:1], vals[:, 1:2],
                                    op=mybir.AluOpType.subtract)
            nc.vector.reciprocal(scr[:, 1:2], scr[:, 0:1])
            nc.scalar.activation(scr[:, 2:3], scr[:, 0:1], AF.Silu)
            g_t = sp.tile([1, 2], F32, tag="g", bufs=2)
            nc.vector.tensor_tensor(g_t[:, 0:1], scr[:, 2:3], scr[:, 1:2],
                                    op=mybir.AluOpType.mult)
            nc.vector.tensor_scalar(
                g_t[:, 1:2], g_t[:, 0:1], -1.0, 1.0,
                op0=mybir.AluOpType.mult, op1=mybir.AluOpType.add)
            ps_g = pp.tile([C, 2], F32, tag="TB", bufs=2, name="ps_g")
            nc.tensor.matmul(ps_g[:], ones[:], g_t[:], start=True, stop=True)
            g_bc = sp.tile([C, 2], F32, tag="gbc", bufs=2)
            nc.vector.tensor_copy(g_bc[:], ps_g[:])
            bexp_sb = sp.tile([C, 2], F32, tag="bexp", bufs=2)
            wks, w2ms = [], []
            for k in range(2):
                iv = nc.values_load(
                    uidx[0:1, k:k + 1], min_val=0, max_val=E - 1,
                    skip_runtime_bounds_check=True)
                wk = sp.tile([C, WCOLS], FP8, tag=f"wk{k}", bufs=2)
                nc.gpsimd.dma_start(wk[:], wexp_d[ds(iv * C, C), :])
                nc.gpsimd.dma_start(bexp_sb[:, k:k + 1], bexp_d[ds(iv * C, C), :])
                w2m = sp.tile([C, 2 * C], BF16, tag=f"w2m{k}", bufs=2)
                nc.vector.tensor_scalar_mul(w2m[:], w2c[2][:], g_bc[:, k:k + 1])
                wks.append(wk)
                w2ms.append(w2m)
            return wks, w2ms, bexp_sb

        def p2_exp(b, rt, sfl, j):
            """Expert DoubleRow matmuls + silu for block pair j."""
            wks, _, bexp_sb = rt
            p0 = (2 * R * j + 1) * HP  # padded start of block 2j
            pes = []
            for k in range(2):
                pe = pp.tile([C, 1024], F32, tag=("TA", "TB")[k], bufs=2,
                             name=f"pe{k}_{j}")
                for blk in range(2):
                    for p, (oa, ob) in enumerate(PAIRS):
                        lhsT = _ap(wks[k][:], p * 2 * C,
                                   [[WCOLS, C], [C, 2], [1, C]])
                        rhs = _ap(m8[b][:], 1 + p0 + blk * NW + oa,
                                  [[MLEN, C], [ob - oa, 2], [1, NW]])
                        nc.tensor.matmul(
                            pe[:, blk * 512:blk * 512 + NW], lhsT, rhs,
                            start=(p == 0), stop=(p == 4), perf_mode=DR)
                pes.append(pe)
            for k in range(2):
                pin = _ap(pes[k], 0, [[1024, C], [512, 2], [1, NW]])
                nc.scalar.activation(
                    sfl[:, k, p0:p0 + 2 * NW], pin, AF.Silu,
                    bias=bexp_sb[:, k:k + 1])

        def p2_tail(b, st, rt, sfl, j, split=False):
            """Gated expert combine + cv2 + output for block pair j.
            split=True shortens the final silu+DMA chain (kernel tail)."""
            ym, _ = st
            _, w2ms, _ = rt
            p0 = (2 * R * j + 1) * HP
            w0 = 2 * j * N
            pos = []
            for mt in range(2):
                po = pp.tile([C, 1024], F32, tag=("TA", "TB")[mt], bufs=2,
                             name=f"po{mt}_{j}")
                for blk in range(2):
                    rhss = [
                        ym[:, 0, w0 + blk * N:w0 + (blk + 1) * N],
                        ym[:, 1, w0 + blk * N:w0 + (blk + 1) * N],
                        _ap(sfl[:], p0 + blk * NW + 1,
                            [[2 * SLEN, C], [HP, R], [1, W]]),
                        _ap(sfl[:], SLEN + p0 + blk * NW + 1,
                            [[2 * SLEN, C], [HP, R], [1, W]]),
                    ]
                    wts = [w2c[0], w2c[1], w2ms[0], w2ms[1]]
                    for ci in range(4):
                        nc.tensor.matmul(
                            po[:, blk * 512:blk * 512 + N],
                            wts[ci][:, mt * C:(mt + 1) * C], rhss[ci],
                            start=(ci == 0), stop=(ci == 3))
                pos.append(po)
            if split:
                for mt in range(2):
                    for blk in range(2):
                        ot = sp.tile([C, N], F32, tag=f"ots{mt}{blk}", bufs=2,
                                     name=f"ots{mt}_{blk}")
                        nc.scalar.activation(
                            ot[:], pos[mt][:, blk * 512:blk * 512 + N],
                            AF.Silu, bias=b2sb[:, mt:mt + 1])
                        eng = nc.sync if (mt + blk) % 2 == 0 else nc.gpsimd
                        eng.dma_start(
                            out_d[b, mt * C:(mt + 1) * C,
                                  w0 + blk * N:w0 + (blk + 1) * N], ot[:])
                return
            for mt in range(2):
                ot = sp.tile([C, 2 * N], F32, tag=f"ot{mt}", bufs=2,
                             name=f"ot{mt}_{j}")
                pin = _ap(pos[mt], 0, [[1024, C], [512, 2], [1, N]])
                nc.scalar.activation(ot[:], pin, AF.Silu, bias=b2sb[:, mt:mt + 1])
                eng = nc.sync if mt == 0 else nc.gpsimd
                eng.dma_start(
                    out_d[b, mt * C:(mt + 1) * C, w0:w0 + 2 * N], ot[:])

        def p2_last_pair(b, st, rt, sfl):
            """Final block pair, processed per block to shorten the kernel
            tail (silu+DMA chain after the last matmul)."""
            ym, _ = st
            wks, w2ms, bexp_sb = rt
            for blk in range(2):
                g = 2 * (NP - 1) + blk
                p0g = (R * g + 1) * HP
                w0g = g * N
                pe = pp.tile([C, 1024], F32, tag="TA", bufs=2, name=f"peL{blk}")
                for k in range(2):
                    for p, (oa, ob) in enumerate(PAIRS):
                        lhsT = _ap(wks[k][:], p * 2 * C,
                                   [[WCOLS, C], [C, 2], [1, C]])
                        rhs = _ap(m8[b][:], 1 + p0g + oa,
                                  [[MLEN, C], [ob - oa, 2], [1, NW]])
                        nc.tensor.matmul(
                            pe[:, k * 512:k * 512 + NW], lhsT, rhs,
                            start=(p == 0), stop=(p == 4), perf_mode=DR)
                for k in range(2):
                    nc.scalar.activation(
                        sfl[:, k, p0g:p0g + NW], pe[:, k * 512:k * 512 + NW],
                        AF.Silu, bias=bexp_sb[:, k:k + 1])
                po = pp.tile([C, 1024], F32, tag="TB", bufs=2, name=f"poL{blk}")
                rhss = [
                    ym[:, 0, w0g:w0g + N],
                    ym[:, 1, w0g:w0g + N],
                    _ap(sfl[:], p0g + 1, [[2 * SLEN, C], [HP, R], [1, W]]),
                    _ap(sfl[:], SLEN + p0g + 1, [[2 * SLEN, C], [HP, R], [1, W]]),
                ]
                wts = [w2c[0], w2c[1], w2ms[0], w2ms[1]]
                for mt in range(2):
                    for ci in range(4):
                        nc.tensor.matmul(
                            po[:, mt * 512:mt * 512 + N],
                            wts[ci][:, mt * C:(mt + 1) * C], rhss[ci],
                            start=(ci == 0), stop=(ci == 3))
                for mt in range(2):
                    ot = sp.tile([C, N], F32, tag=f"otL{mt}", bufs=2,
                                 name=f"otL{mt}_{blk}")
                    nc.scalar.activation(
                        ot[:], po[:, mt * 512:mt * 512 + N], AF.Silu,
                        bias=b2sb[:, mt:mt + 1])
                    eng = nc.sync if mt == 0 else nc.gpsimd
                    eng.dma_start(
                        out_d[b, mt * C:(mt + 1) * C, w0g:w0g + N], ot[:])

        def p2_phase(b, st, rt, sfl, j0=0, lag=1, inject_rt=None, last=False,
                     post=None):
            rt_out = None
            for j in range(j0, NP):
                p2_exp(b, rt, sfl, j)
                if j == j0 and post is not None:
                    post()
                if j - lag >= 0:
                    p2_tail(b, st, rt, sfl, j - lag)
                if j == 3 and inject_rt is not None:
                    rt_out = routing(*inject_rt)
            for t in range(NP - lag, NP):
                p2_tail(b, st, rt, sfl, t, split=(last and t == NP - 1))
            return rt_out

        w2c = b2sb = wrtsb = brsb = ones = None
        for _rep in range(reps):
            st0 = p1_state(0)
            p1_pairs(0, st0, 0, 1)
            if w2c is None:
                w2c, b2sb, wrtsb, brsb, ones = setup_tail()
            p1_pairs(0, st0, 1, NP)
            st1 = p1_state(1)
            p1_pairs(1, st1, 0, 3)
            rt0 = routing(0, st0)
            p1_pairs(1, st1, 3, NP)
            sfl0 = sp.tile([C, 2, SLEN], BF16, tag="sfl", bufs=2, name="s0")
            rt1 = p2_phase(0, st0, rt0, sfl0, inject_rt=(1, st1))
            sfl1 = sp.tile([C, 2, SLEN], BF16, tag="sfl", bufs=2, name="s1")
            p2_phase(1, st1, rt1, sfl1, last=(_rep == reps - 1))


def _prep_inputs(x, W_cv1, b_cv1, W_r, b_r, W_exp, b_exp, W_cv2, b_cv2):
    """Host-side layout/dtype prep shared by kernel() and test drivers."""
    BF = ml_dtypes.bfloat16
    F8 = ml_dtypes.float8_e4m3
    x = np.ascontiguousarray(np.asarray(x, dtype=np.float32))
    w1t = np.ascontiguousarray(W_cv1[:, :, 0, 0].T).astype(BF)
    w2t = np.ascontiguousarray(W_cv2[:, :, 0, 0].T).astype(BF)
    wrt = np.ascontiguousarray(W_r.T).astype(np.float32)
    blob = np.zeros((E, C, 5, 2, C), np.float32)
    for p, (ta, tb) in enumerate(TAP_PAIRS):
        blob[:, :, p, 0, :] = W_exp[:, :, :, ta[0], ta[1]].transpose(0, 2, 1)
        if tb is not None:
            blob[:, :, p, 1, :] = W_exp[:, :, :, tb[0], tb[1]].transpose(0, 2, 1)
    wexp8 = np.ascontiguousarray(blob.reshape(E * C, WCOLS)).astype(F8)
    shared = {
        "w1t": w1t,
        "b1": np.asarray(b_cv1, np.float32).reshape(-1, 1),
        "wrt": wrt,
        "br": np.asarray(b_r, np.float32).reshape(-1, 1),
        "wexp8": wexp8,
        "bexp": np.asarray(b_exp, np.float32).reshape(E * C, 1),
        "w2t": w2t,
        "b2": np.asarray(b_cv2, np.float32).reshape(-1, 1),
    }
    xb = x.reshape(B, C1, S).astype(BF)
    in_maps = [
        {**shared, "xb": np.ascontiguousarray(xb[i * BPC:(i + 1) * BPC])}
        for i in range(N_CORES)
    ]
    return in_maps


def kernel(x, W_cv1, b_cv1, W_r, b_r, W_exp, b_exp, W_cv2, b_cv2):
    args = [np.asarray(a, dtype=np.float32) for a in
            (x, W_cv1, b_cv1, W_r, b_r, W_exp, b_exp, W_cv2, b_cv2)]
    if "nc" not in _cache:
        _cache["nc"] = _build_program()
    nc = _cache["nc"]
    in_maps = _prep_inputs(*args)
    res = run_bass_kernel_spmd(nc, in_maps, core_ids=list(range(N_CORES)))
    _cache["last_results"] = res
    out = np.concatenate([res.results[i]["out"] for i in range(N_CORES)], axis=0)
    return out.reshape(B, 2 * C, H, W)
